# revision 1
# baseline (speedup 1.0000x reference)
"""DGMC (deep graph matching consensus) Bass kernel for 8 Trainium2 NeuronCores.

Problem (see reference):
  B=4 graph pairs, N=1024 nodes/graph, F_IN=128, F_HID=256, R=16, 2 steps,
  E=65536 random edges per graph side (edges span the whole 4096-node set).

  h_s = relu(((I+A_s) @ x_s) @ W1 + b1);  same for h_t
  S_hat = h_s @ h_t^T  per batch                      [B, N, N]
  S_0 = softmax(S_hat)
  2x: S = softmax(S_hat); r_t = S^T @ r_s
      o_s = relu(((I+A_s) @ r_s) @ W2 + b2); o_t likewise with r_t
      S_hat += mlp(o_s[s] - o_t[t]) pointwise over [B, N, N, R]
  S_L = softmax(S_hat)

Sharding: core c = 2*b + h handles batch b, s-half h (512 similarity rows).
All per-core variation is carried in the DATA (sliced adjacency etc), the
program is identical on all 8 cores.  The only cross-core communication is
one AllGather of partial r_t (each core sums S^T r_s over its 512 s-rows)
per step.

Precision strategy: every value that feeds softmax needs ~1e-5 relative
accuracy (S_hat magnitudes reach ~1e3).  Integer-valued adjacency matrices
are exact in bf16; float operands are split hi/lo into two bf16 matmuls
(exact to ~2^-17).  Small dense matmuls (W1/W2/Wm1 chains, S_hat, r_t) run
in true fp32 on the PE (4 cycles/row).  The big [N_s, N_t, R] MLP runs as:
  first-mm (bf16, K=97) per (j = 8-row s-block, t-block of 512):
     psum[(s8,k), t] = A'[s8,k] - P'[t,k],  A' = o_s@Wm1' + bm1'
     lhsT cols = (s8*16+k); rows 0..15 / 32..47 = delta_{row,k} patterns
     pairing rhs rows = (-o_t@Wm1')^T hi / lo in natural [16, t] layout;
     rows 64 / 96 = A' flat hi / lo pairing ones rows.  (32-aligned row
     bases because engine partition windows must start at 0/32/64/96.)
  relu (alternating ACT/DVE) -> H as float32r in SBUF
  second-mm (f32r): psum2[s, t] += Z_j^T @ H_j with Z_j the shifted
     sign(Wm2) selection matrix - the k-reduction runs on the PE; a single
     DVE add folds psum2 into S_hat.  Wm1' = Wm1 * |Wm2|; bm2 is dropped
     (softmax is shift-invariant).
"""

import numpy as np
import ml_dtypes

import concourse.bass as bass
import concourse.bacc as bacc
import concourse.tile as tile
from concourse import mybir
from concourse import bass_utils

F32 = mybir.dt.float32
BF16 = mybir.dt.bfloat16

B, N, F_IN, F_HID, R, NUM_STEPS, E = 4, 1024, 128, 256, 16, 2, 65536
NNODE = B * N
NCHUNK = NNODE // 128   # 32 source chunks
NCORES = 8
SH = N // 2             # 512 s rows per core
NSC = SH // 128         # 4 s-chunks per core
TK = N * R              # 16384 free columns in (t, k) layout
AF = mybir.ActivationFunctionType


def _bf_split(x):
    hi = np.asarray(x, dtype=np.float32).astype(ml_dtypes.bfloat16)
    lo = (np.asarray(x, dtype=np.float32) - hi.astype(np.float32)).astype(
        ml_dtypes.bfloat16)
    return hi, lo


def _chunk_major(x):
    """[4096, W] -> [128, 32*W] with col (c*W + w) = x[c*128 + p, w]."""
    w = x.shape[1]
    return np.ascontiguousarray(
        x.reshape(NCHUNK, 128, w).transpose(1, 0, 2).reshape(128, NCHUNK * w))


def _adjT_plus_I(edge_index):
    """(A^T + I) as float32 (exact small ints); A[dst, src] = edge count."""
    src = np.asarray(edge_index[0], dtype=np.int64)
    dst = np.asarray(edge_index[1], dtype=np.int64)
    flat = src * NNODE + dst
    cnt = np.bincount(flat, minlength=NNODE * NNODE).astype(np.float32)
    at = cnt.reshape(NNODE, NNODE)
    at[np.arange(NNODE), np.arange(NNODE)] += 1.0
    return at


_BUILD_CACHE = {}


def _build(kp, bm2val, mock_cc=False):
    key = (kp, float(bm2val), mock_cc)
    if key in _BUILD_CACHE:
        return _BUILD_CACHE[key]

    nc = bacc.Bacc("TRN2", target_bir_lowering=False, debug=False,
                   num_devices=NCORES)

    # ---- DRAM I/O (per-core data) ----
    d_att = nc.dram_tensor("att", [128, NCHUNK * N], BF16, kind="ExternalInput")
    d_ats = nc.dram_tensor("ats", [NCHUNK, 128, SH], BF16, kind="ExternalInput")
    d_xshi = nc.dram_tensor("xshi", [128, NCHUNK * F_IN], BF16, kind="ExternalInput")
    d_xslo = nc.dram_tensor("xslo", [128, NCHUNK * F_IN], BF16, kind="ExternalInput")
    d_xthi = nc.dram_tensor("xthi", [128, NCHUNK * F_IN], BF16, kind="ExternalInput")
    d_xtlo = nc.dram_tensor("xtlo", [128, NCHUNK * F_IN], BF16, kind="ExternalInput")
    # packed o_s lhsT: per chunk, 112 cols = step0-hi@0, step0-lo@32,
    # step1-hi@64, step1-lo@96 (zeros between) -> one M=112 matmul per chunk
    d_rsnp = nc.dram_tensor("rsnp", [128, NCHUNK * 112], BF16,
                            kind="ExternalInput")
    d_rsown = nc.dram_tensor("rsown", [128, NUM_STEPS * NSC * R], F32,
                             kind="ExternalInput")
    d_w1 = nc.dram_tensor("w1", [F_IN, F_HID], F32, kind="ExternalInput")
    d_b1c = nc.dram_tensor("b1c", [128, 2], F32, kind="ExternalInput")
    d_w2 = nc.dram_tensor("w2", [R, R], F32, kind="ExternalInput")
    d_b2c = nc.dram_tensor("b2c", [R, 1], F32, kind="ExternalInput")
    d_wm1pA = nc.dram_tensor("wm1pA", [R, R], F32, kind="ExternalInput")
    d_wm1pP = nc.dram_tensor("wm1pP", [R, R], F32, kind="ExternalInput")
    d_bm1pc = nc.dram_tensor("bm1pc", [R, 1], F32, kind="ExternalInput")
    d_indb = nc.dram_tensor("indb", [64, 64 * 128], BF16, kind="ExternalInput")
    d_zbig = nc.dram_tensor("zbig", [128, 248], mybir.dt.float32r,
                            kind="ExternalInput")
    d_bm1pb = nc.dram_tensor("bm1pb", [128, R], F32, kind="ExternalInput")
    d_abounce = nc.dram_tensor("abounce", [NUM_STEPS, 2, SH * R], BF16)

    d_s0 = nc.dram_tensor("s0o", [SH, N], F32, kind="ExternalOutput")
    d_sl = nc.dram_tensor("slo", [SH, N], F32, kind="ExternalOutput")

    # collective buffers (per step); AllGather concatenates along dim 0
    d_ccin = [nc.dram_tensor(f"ccin{i}", [128, 128], F32) for i in range(NUM_STEPS)]
    d_ccout = [nc.dram_tensor(f"ccout{i}", [NCORES * 128, 128], F32,
                              addr_space="Shared") for i in range(NUM_STEPS)]

    with tile.TileContext(nc) as tc:
        # ---------- resident tiles ----------
        with tc.tile_pool(name="res", bufs=1) as res:
            att = res.tile([128, NCHUNK * N], BF16)
            lwb = res.tile([128, 64 * 128], BF16)
            nc.gpsimd.memset(lwb[64:128, :], 0.0)
            rhb = res.tile([128, N], BF16)
            nc.gpsimd.memset(rhb[:], 0.0)
            nc.gpsimd.memset(rhb[64:65, :], 1.0)
            nc.gpsimd.memset(rhb[96:97, :], 1.0)
            zbig = res.tile([128, 248], mybir.dt.float32r)
            bm1pb = res.tile([128, R], F32)
            rsnp = res.tile([128, NCHUNK * 112], BF16)
            rt48 = res.tile([128, NCHUNK * 48], BF16)
            nc.gpsimd.memset(rt48[:], 0.0)
            rsown = res.tile([128, NUM_STEPS * NSC * R], F32)
            w1 = res.tile([F_IN, F_HID], F32)
            b1c = res.tile([128, 2], F32)
            w2 = res.tile([R, R], F32)
            b2c = res.tile([R, 1], F32)
            wm1pA = res.tile([R, R], F32)
            wm1pP = res.tile([R, R], F32)
            bm1pc = res.tile([R, 1], F32)


            shat = res.tile([128, NSC * N], F32)      # [128, (sc, t)]

            smp = res  # softmax tiles live in the resident pool
            out_pool_ctx = tc.tile_pool(name="outp", bufs=2)
            out_pool = out_pool_ctx.__enter__()
            pexp = smp.tile([128, NSC * N], F32)   # exp(shat - max)
            rzt = [dict(nm=smp.tile([128, 1], F32, tag=f"nm{sc}", name=f"nm{sc}"),
                        z=smp.tile([128, 1], F32, tag=f"z{sc}", name=f"z{sc}"),
                        rz=smp.tile([128, 1], F32, tag=f"rz{sc}", name=f"rz{sc}"))
                   for sc in range(NSC)]
            emit_out = []

            def flush_out():
                while emit_out:
                    sc, ssl, rz, out_dram = emit_out.pop(0)
                    so = out_pool.tile([128, N], F32, tag="so", name="so")
                    nc.vector.tensor_scalar_mul(so[:], pexp[:, ssl],
                                                rz[:, 0:1])
                    nc.sync.dma_start(
                        out_dram.ap()[sc * 128:(sc + 1) * 128, :], so[:])

            def softmax_sc(rz_tiles, sc, out_dram, defer=True):
                ssl = slice(sc * N, (sc + 1) * N)
                nm = rz_tiles[sc]["nm"]
                z = rz_tiles[sc]["z"]
                rz = rz_tiles[sc]["rz"]
                nc.vector.reduce_max(nm[:], shat[:, ssl], negate=True,
                                     axis=mybir.AxisListType.X)
                nc.scalar.activation(pexp[:, ssl], shat[:, ssl], AF.Exp,
                                     bias=nm[:, 0:1], accum_out=z[:, 0:1])
                nc.vector.reciprocal(rz[:], z[:])
                if out_dram is not None:
                    if defer:
                        emit_out.append((sc, ssl, rz, out_dram))
                    else:
                        so = out_pool.tile([128, N], F32, tag="so", name="so")
                        nc.vector.tensor_scalar_mul(so[:], pexp[:, ssl],
                                                    rz[:, 0:1])
                        nc.sync.dma_start(
                            out_dram.ap()[sc * 128:(sc + 1) * 128, :], so[:])

            # ---------- phase A: psi_1 aggregates + o_s chains ----------
            with tc.tile_pool(name="pA", bufs=1, space="PSUM") as pA, \
                 tc.tile_pool(name="sA", bufs=6) as sA, \
                 tc.tile_pool(name="xA", bufs=1) as xA, \
                 tc.tile_pool(name="wA", bufs=1) as wA:
                xshi = xA.tile([128, NCHUNK * F_IN], BF16)
                xslo = xA.tile([128, NCHUNK * F_IN], BF16)
                xthi = xA.tile([128, NCHUNK * F_IN], BF16)
                xtlo = xA.tile([128, NCHUNK * F_IN], BF16)
                nc.sync.dma_start(xshi[:], d_xshi.ap())
                nc.sync.dma_start(rsnp[:], d_rsnp.ap())
                # remaining loads are spread into the chunk loop below so
                # their HWDGE descriptor slots don't delay the first chunks
                deferred_loads = [
                    (xthi, d_xthi), (xtlo, d_xtlo), (rsown, d_rsown),
                    (w1, d_w1), (b1c, d_b1c), (w2, d_w2), (b2c, d_b2c),
                    (wm1pA, d_wm1pA), (wm1pP, d_wm1pP), (bm1pc, d_bm1pc),
                    (bm1pb, d_bm1pb), (zbig, d_zbig),
                ]
                agg_s = pA.tile([128, SH], F32)
                agg_t0 = pA.tile([128, 512], F32)
                agg_t1 = pA.tile([128, 512], F32)
                aggo = pA.tile([112, SH], F32)
                # att-independent matmuls first: the PE is in-order, so an
                # agg_t matmul waiting on the 8MB att DMA would stall the
                # whole PE stream behind it.
                for c in range(NCHUNK):
                    ats_c = sA.tile([128, SH], BF16)
                    nc.sync.dma_start(ats_c[:], d_ats.ap()[c])
                    # interleave att column loads so they finish just before
                    # the second loop needs them (serial DMA model)
                    nc.sync.dma_start(att[:, c * N:(c + 1) * N],
                                      d_att.ap()[:, c * N:(c + 1) * N])
                    if c == 0:
                        # lo operand consumed later in this very iteration:
                        # its DMA must precede the matmuls in trace order
                        # (Tile deps are trace-order based)
                        nc.sync.dma_start(xslo[:], d_xslo.ap())
                    if c >= 1 and deferred_loads:
                        t_, d_ = deferred_loads.pop(0)
                        nc.sync.dma_start(t_[:], d_.ap())
                    if c == NCHUNK - 1:
                        nc.sync.dma_start(lwb[0:64, :], d_indb.ap())
                    st, sp = (c == 0), (c == NCHUNK - 1)
                    xs_sl = slice(c * F_IN, (c + 1) * F_IN)
                    nc.tensor.matmul(agg_s[:], xshi[:, xs_sl], ats_c[:],
                                     start=st, stop=False)
                    nc.tensor.matmul(agg_s[:], xslo[:, xs_sl], ats_c[:],
                                     start=False, stop=sp)
                    nc.tensor.matmul(aggo[:],
                                     rsnp[:, c * 112:(c + 1) * 112],
                                     ats_c[:], start=st, stop=sp)
                for c in range(NCHUNK):
                    st, sp = (c == 0), (c == NCHUNK - 1)
                    xs_sl = slice(c * F_IN, (c + 1) * F_IN)
                    att0 = att[:, c * N: c * N + 512]
                    att1 = att[:, c * N + 512: (c + 1) * N]
                    nc.tensor.matmul(agg_t0[:], xthi[:, xs_sl], att0,
                                     start=st, stop=False)
                    nc.tensor.matmul(agg_t0[:], xtlo[:, xs_sl], att0,
                                     start=False, stop=sp)
                    nc.tensor.matmul(agg_t1[:], xthi[:, xs_sl], att1,
                                     start=st, stop=False)
                    nc.tensor.matmul(agg_t1[:], xtlo[:, xs_sl], att1,
                                     start=False, stop=sp)

                # h^T tiles: h = relu(v @ W1 + b1), v^T held in psum aggs
                h_sT = xA.tile([128, 2 * SH], F32)    # [128, (fc, s)]
                h_tT = xA.tile([128, 2 * N], F32)     # [128, (fc, t)]
                vs = wA.tile([128, SH], F32)
                nc.scalar.copy(vs[:], agg_s[:])
                vt0 = wA.tile([128, 512], F32)
                nc.scalar.copy(vt0[:], agg_t0[:])
                vt1 = wA.tile([128, 512], F32)
                nc.scalar.copy(vt1[:], agg_t1[:])
                for fc in range(2):
                    w1sl = w1[:, fc * 128:(fc + 1) * 128]
                    ph = pA.tile([128, 512], F32, tag="ph")
                    nc.tensor.matmul(ph[:], w1sl, vs[:], start=True, stop=True)
                    nc.scalar.activation(h_sT[:, fc * SH:(fc + 1) * SH], ph[:],
                                         AF.Relu, bias=b1c[:, fc:fc + 1])
                    ph2 = pA.tile([128, 512], F32, tag="ph")
                    nc.tensor.matmul(ph2[:], w1sl, vt0[:], start=True, stop=True)
                    nc.scalar.activation(h_tT[:, fc * N: fc * N + 512], ph2[:],
                                         AF.Relu, bias=b1c[:, fc:fc + 1])
                    ph3 = pA.tile([128, 512], F32, tag="ph")
                    nc.tensor.matmul(ph3[:], w1sl, vt1[:], start=True, stop=True)
                    nc.scalar.activation(h_tT[:, fc * N + 512:(fc + 1) * N],
                                         ph3[:], AF.Relu, bias=b1c[:, fc:fc + 1])

                # o_s chains for both steps; A' = o_s@Wm1' + bm1' in
                # [s, k] layout, bf16 hi/lo bounced through DRAM so the
                # s-major flattened rows can be DMAd into lwb rows 64/96.
                for i in range(NUM_STEPS):
                    # DVE tensor-tensor needs both inputs at the same
                    # partition base: stage the packed psum slices via ACT
                    agh = wA.tile([16, SH], F32, tag="agh")
                    nc.scalar.copy(agh[:], aggo[64 * i:64 * i + 16, :])
                    agl = wA.tile([16, SH], F32, tag="agl")
                    nc.scalar.copy(agl[:], aggo[64 * i + 32:64 * i + 48, :])
                    pre = wA.tile([16, SH], F32, tag="pre")
                    nc.vector.tensor_add(pre[:], agh[:], agl[:])
                    pz = pA.tile([16, SH], F32, tag="pz")
                    nc.tensor.matmul(pz[:], w2[:], pre[:], start=True, stop=True)
                    osT = wA.tile([16, SH], F32, tag="osT")
                    nc.scalar.activation(osT[:], pz[:], AF.Relu,
                                         bias=b2c[:, 0:1])
                    for scc in range(NSC):
                        pa = pA.tile([128, R], F32, tag="ph", name="paB")
                        nc.tensor.matmul(pa[:],
                                         osT[:, scc * 128:(scc + 1) * 128],
                                         wm1pA[:], start=True, stop=True)
                        ap = wA.tile([128, R], F32, tag="apB")
                        nc.vector.tensor_add(ap[:], pa[:], bm1pb[:])
                        aphi = wA.tile([128, R], BF16, tag="aphi")
                        nc.vector.tensor_copy(aphi[:], ap[:])
                        aphi32 = wA.tile([128, R], F32, tag="aphi32")
                        nc.vector.tensor_copy(aphi32[:], aphi[:])
                        aplo32 = wA.tile([128, R], F32, tag="aplo32")
                        nc.vector.tensor_sub(aplo32[:], ap[:], aphi32[:])
                        aplo = wA.tile([128, R], BF16, tag="aplo")
                        nc.vector.tensor_copy(aplo[:], aplo32[:])
                        csl = slice(scc * 128 * R, (scc + 1) * 128 * R)
                        nc.sync.dma_start(d_abounce.ap()[i, 0, csl], aphi[:])
                        nc.sync.dma_start(d_abounce.ap()[i, 1, csl], aplo[:])

                # ---------- S_hat ----------
                for sc in range(NSC):
                    for tb in range(2):
                        ps = pA.tile([128, 512], F32, tag="pS")
                        for fc in range(2):
                            nc.tensor.matmul(
                                ps[:],
                                h_sT[:, fc * SH + sc * 128: fc * SH + (sc + 1) * 128],
                                h_tT[:, fc * N + tb * 512: fc * N + (tb + 1) * 512],
                                start=(fc == 0), stop=(fc == 1))
                        nc.scalar.copy(
                            shat[:, sc * N + tb * 512: sc * N + (tb + 1) * 512],
                            ps[:])
                    softmax_sc(rzt, sc, d_s0)   # S_0 softmax, overlapped

            # ---------- steps ----------

            with tc.tile_pool(name="step", bufs=1) as stp, \
                 tc.tile_pool(name="pS", bufs=1, space="PSUM") as pSm, \
                 tc.tile_pool(name="pD", bufs=3, space="PSUM") as pD, \
                 tc.tile_pool(name="pD2", bufs=1, space="PSUM") as pD2, \
                 tc.tile_pool(name="hD", bufs=4) as hD:
                for i in range(NUM_STEPS):
                    # softmax for this step was emitted by the previous
                    # D-phase tail (or above for step 0)
                    # r_t partial: lhsT = pexp slice, rhs = r_s * (1/Z)
                    rsp = stp.tile([128, NSC * R], F32, tag="rsp")
                    for sc in range(NSC):
                        nc.vector.tensor_scalar_mul(
                            rsp[:, sc * R:(sc + 1) * R],
                            rsown[:, i * NSC * R + sc * R:
                                  i * NSC * R + (sc + 1) * R],
                            rzt[sc]["rz"][:, 0:1])
                    prt = pSm.tile([128, 128], F32, tag="prt")
                    for tcn in range(8):
                        for sc in range(NSC):
                            nc.tensor.matmul(
                                prt[:, tcn * R:(tcn + 1) * R],
                                pexp[:, sc * N + tcn * 128:
                                     sc * N + (tcn + 1) * 128],
                                rsp[:, sc * R:(sc + 1) * R],
                                start=(sc == 0), stop=(sc == NSC - 1))
                    ccs = stp.tile([128, 128], F32, tag="ccs")
                    nc.vector.tensor_copy(ccs[:], prt[:])
                    nc.sync.dma_start(d_ccin[i].ap(), ccs[:])
                    if mock_cc:
                        # stand-in for the AllGather so TimelineSim (single
                        # core, no collectives) can cost the kernel
                        for cg in range(NCORES):
                            nc.sync.dma_start(
                                d_ccout[i].ap()[cg * 128:(cg + 1) * 128, :],
                                d_ccin[i].ap())
                    else:
                        nc.gpsimd.collective_compute(
                            "AllGather", mybir.AluOpType.bypass,
                            replica_groups=[list(range(NCORES))],
                            ins=[d_ccin[i].ap()], outs=[d_ccout[i].ap()])
                    flush_out()
                    gath = stp.tile([128, NCORES * 128], F32, tag="gath")
                    nc.sync.dma_start(
                        gath[:].rearrange("p (c x) -> p c x", c=NCORES),
                        d_ccout[i].ap().rearrange("(c p) x -> p c x", c=NCORES))
                    rt = stp.tile([128, 512], F32, tag="rt")
                    g4 = gath[:].rearrange("p (b h x) -> p b h x", b=4, h=2)
                    nc.vector.tensor_add(
                        rt[:].rearrange("p (b x) -> p b x", b=4),
                        g4[:, :, 0, :], g4[:, :, 1, :])
                    r48 = rt48[:].rearrange("p (c w) -> p c w", w=48)
                    rtv = rt[:].rearrange("p (c r) -> p c r", r=R)
                    nc.vector.tensor_copy(r48[:, :, 0:16], rtv)
                    rthi32 = stp.tile([128, 512], F32, tag="rthi32")
                    nc.vector.tensor_copy(
                        rthi32[:].rearrange("p (c r) -> p c r", r=R),
                        r48[:, :, 0:16])
                    rtlo32 = stp.tile([128, 512], F32, tag="rtlo32")
                    nc.vector.tensor_sub(rtlo32[:], rt[:], rthi32[:])
                    nc.vector.tensor_copy(
                        r48[:, :, 32:48],
                        rtlo32[:].rearrange("p (c r) -> p c r", r=R))

                    # o_t chain
                    otT = stp.tile([16, N], F32, tag="otT")
                    for tb in range(2):
                        pago = pSm.tile([48, 512], F32, tag="pago")
                        for c in range(NCHUNK):
                            attb = att[:, c * N + tb * 512:
                                       c * N + (tb + 1) * 512]
                            nc.tensor.matmul(pago[:],
                                             rt48[:, c * 48:(c + 1) * 48],
                                             attb, start=(c == 0),
                                             stop=(c == NCHUNK - 1))
                        plo = stp.tile([16, 512], F32, tag="plo")
                        nc.scalar.copy(plo[:], pago[32:48, :])
                        pre_t = stp.tile([16, 512], F32, tag="pre_t")
                        nc.vector.tensor_add(pre_t[:], pago[0:16, :], plo[:])
                        pzt = pSm.tile([16, 512], F32, tag="pzt")
                        nc.tensor.matmul(pzt[:], w2[:], pre_t[:], start=True,
                                         stop=True)
                        nc.scalar.activation(otT[:, tb * 512:(tb + 1) * 512],
                                             pzt[:], AF.Relu, bias=b2c[:, 0:1])
                    ppT = stp.tile([16, N], F32, tag="ppT")
                    pphi32 = stp.tile([16, N], F32, tag="pphi32")
                    pplo32 = stp.tile([16, N], F32, tag="pplo32")
                    for tb in range(2):
                        tsl = slice(tb * 512, (tb + 1) * 512)
                        ppp = pSm.tile([16, 512], F32, tag="pzt")
                        nc.tensor.matmul(ppp[:], wm1pP[:], otT[:, tsl],
                                         start=True, stop=True)
                        nc.scalar.copy(ppT[:, tsl], ppp[:])
                        # bf16 hi/lo per t-half so tb=0 D-tiles start early
                        nc.vector.tensor_copy(rhb[0:16, tsl], ppT[:, tsl])
                        nc.vector.tensor_copy(pphi32[:, tsl], rhb[0:16, tsl])
                        nc.vector.tensor_sub(pplo32[:, tsl], ppT[:, tsl],
                                             pphi32[:, tsl])
                        nc.vector.tensor_copy(rhb[32:48, tsl], pplo32[:, tsl])
                    # A' rows for this step into lwb rows 64 / 96
                    nc.sync.dma_start(lwb[64:65, :], d_abounce.ap()[i, 0, :])
                    nc.sync.dma_start(lwb[96:97, :], d_abounce.ap()[i, 1, :])

                    # ---------- D-phase ----------
                    # first-mm -> relu -> second-mm (k-reduction on PE via
                    # the shifted sign matrix zbig), one DVE add per tile.
                    # software-pipelined: emit first-mm(j+1) before
                    # second-mm(j) so the in-order PE never waits on the
                    # relu between them.  The pipeline runs across the whole
                    # (sc, tb) space; second-mms accumulate into per-(sc,tb)
                    # psum tiles.
                    tiles = [(sc, tb, j8) for tb in range(2)
                             for sc in range(NSC) for j8 in range(16)]
                    ps2m = {}
                    pending = []

                    def emit_second(sc, tb, j8, ht):
                        nc.tensor.matmul(
                            ps2m[(sc, tb)][:],
                            zbig[:, 120 - 8 * j8: 248 - 8 * j8], ht[:],
                            start=(j8 == 0), stop=(j8 == 15))
                        if j8 == 15:
                            ssl = slice(sc * N + tb * 512,
                                        sc * N + (tb + 1) * 512)
                            nc.vector.tensor_add(shat[:, ssl], shat[:, ssl],
                                                 ps2m[(sc, tb)][:])
                            if tb == 1:
                                # this sc's S_hat is final for the step:
                                # overlap the next softmax with the rest of
                                # the D-phase (engines run their streams in
                                # order, so this must be emitted here)
                                if i + 1 < NUM_STEPS:
                                    softmax_sc(rzt, sc, None)
                                else:
                                    softmax_sc(rzt, sc, d_sl, defer=False)

                    for idx, (sc, tb, j8) in enumerate(tiles):
                        if (sc, tb) not in ps2m:
                            ps2m[(sc, tb)] = pD2.tile([128, 512], F32,
                                                      tag=f"ps2_{sc % 2}",
                                                      name=f"ps2_{sc}_{tb}")
                        j = sc * 16 + j8
                        pd = pD.tile([128, 512], F32, tag="pd", name="pd")
                        nc.tensor.matmul(
                            pd[:],
                            lwb[0:97, j * 128:(j + 1) * 128],
                            rhb[0:97, tb * 512:(tb + 1) * 512],
                            start=True, stop=True)
                        ht = hD.tile([128, 512], mybir.dt.float32r,
                                     tag="ht", name="ht")
                        if idx % 2 == 0:
                            nc.scalar.activation(ht[:], pd[:], AF.Relu)
                        else:
                            nc.vector.tensor_scalar_max(ht[:], pd[:], 0.0)
                        pending.append((sc, tb, j8, ht))
                        if len(pending) > 1:
                            emit_second(*pending.pop(0))
                    while pending:
                        emit_second(*pending.pop(0))

                flush_out()
            out_pool_ctx.__exit__(None, None, None)
            out_pool_ctx.__exit__(None, None, None)

    nc.compile()
    _BUILD_CACHE[key] = nc
    return nc


def _prep_core_inputs(core, shared, per_graph):
    """Assemble the in_map for one core."""
    b, h = core // 2, core % 2
    at_s_T, at_t_T = per_graph["ats"], per_graph["att"]
    tcols = slice(b * N, (b + 1) * N)
    scols = slice(b * N + h * SH, b * N + h * SH + SH)
    att_cm = _chunk_major(
        np.ascontiguousarray(at_t_T[:, tcols])).astype(ml_dtypes.bfloat16)
    ats_chunks = np.ascontiguousarray(
        at_s_T[:, scols].reshape(NCHUNK, 128, SH)).astype(ml_dtypes.bfloat16)
    r_s = shared["r_s_steps"]   # [2, B, N, R]
    rs_own = np.zeros((128, NUM_STEPS * NSC * R), np.float32)
    for i in range(NUM_STEPS):
        blk = r_s[i, b, h * SH: h * SH + SH, :].reshape(NSC, 128, R)
        rs_own[:, i * NSC * R:(i + 1) * NSC * R] = (
            blk.transpose(1, 0, 2).reshape(128, NSC * R))
    return dict(
        att=att_cm,
        ats=ats_chunks,
        xshi=shared["xshi"], xslo=shared["xslo"],
        xthi=shared["xthi"], xtlo=shared["xtlo"],
        rsnp=shared["rsnp"],
        rsown=rs_own,
        w1=shared["w1"], b1c=shared["b1c"], w2=shared["w2"],
        b2c=shared["b2c"], wm1pA=shared["wm1pA"], wm1pP=shared["wm1pP"],
        bm1pc=shared["bm1pc"], indb=shared["indb"],
        zbig=shared["zbig"], bm1pb=shared["bm1pb"],
    )


def prepare(x_s, edge_index_s, batch_s, x_t, edge_index_t, batch_t,
            r_s_steps, W1, b1, W2, b2, Wm1, bm1, Wm2, bm2):
    """Host-side preprocessing shared by kernel() and the sim harness."""
    x_s = np.asarray(x_s, np.float32)
    x_t = np.asarray(x_t, np.float32)
    r_s_steps = np.asarray(r_s_steps, np.float32)
    W1 = np.asarray(W1, np.float32)
    b1 = np.asarray(b1, np.float32)
    W2 = np.asarray(W2, np.float32)
    b2 = np.asarray(b2, np.float32)
    Wm1 = np.asarray(Wm1, np.float32)
    bm1 = np.asarray(bm1, np.float32)
    Wm2 = np.asarray(Wm2, np.float32).reshape(R)
    bm2 = float(np.asarray(bm2, np.float32).reshape(()))

    bm2 = 0.0  # uniform shift of S_hat; softmax outputs are invariant to it
    # |Wm2|-folded MLP weights; signs live in the zbig reduction matrix
    signs = np.where(Wm2 >= 0, 1.0, -1.0)
    kp = 0  # unused; program is input-shape-only
    wm1p = Wm1 * np.abs(Wm2)[None, :]
    bm1p = bm1 * np.abs(Wm2)

    xshi, xslo = _bf_split(x_s)
    xthi, xtlo = _bf_split(x_t)
    rs_flat = r_s_steps.reshape(NUM_STEPS, NNODE, R)
    rsnp = np.zeros((128, NCHUNK, 112), np.float32)
    for i in range(NUM_STEPS):
        cm = _chunk_major(rs_flat[i]).reshape(128, NCHUNK, R)
        hi, lo = _bf_split(cm)
        rsnp[:, :, 64 * i:64 * i + 16] = hi.astype(np.float32)
        rsnp[:, :, 64 * i + 32:64 * i + 48] = lo.astype(np.float32)
    rsnp = rsnp.reshape(128, NCHUNK * 112).astype(ml_dtypes.bfloat16)

    indb = np.zeros((64, 64 * 128), np.float32)
    eye_tiled = np.tile(np.eye(R, dtype=np.float32), (1, 8 * 64))  # [16, 8192]
    indb[0:16, :] = eye_tiled
    indb[32:48, :] = eye_tiled
    zbig = np.zeros((128, 248), np.float32)
    for s8 in range(8):
        for k in range(R):
            zbig[s8 * R + k, 120 + s8] = signs[k]

    shared = dict(
        r_s_steps=r_s_steps,
        xshi=_chunk_major(xshi.astype(np.float32)).astype(ml_dtypes.bfloat16),
        xslo=_chunk_major(xslo.astype(np.float32)).astype(ml_dtypes.bfloat16),
        xthi=_chunk_major(xthi.astype(np.float32)).astype(ml_dtypes.bfloat16),
        xtlo=_chunk_major(xtlo.astype(np.float32)).astype(ml_dtypes.bfloat16),
        rsnp=rsnp,
        w1=W1, b1c=b1.reshape(2, 128).T.copy(),
        w2=W2, b2c=b2.reshape(R, 1),
        wm1pA=wm1p, wm1pP=-wm1p, bm1pc=bm1p.reshape(R, 1),
        indb=indb.astype(ml_dtypes.bfloat16),
        zbig=zbig,
        bm1pb=np.tile(bm1p[None, :], (128, 1)).astype(np.float32),
    )
    per_graph = dict(ats=_adjT_plus_I(edge_index_s),
                     att=_adjT_plus_I(edge_index_t))
    in_maps = [_prep_core_inputs(c, shared, per_graph) for c in range(NCORES)]
    return in_maps, kp, bm2


def assemble(results):
    """Stack per-core [SH, N] outputs into full [B*N, N] S_0 / S_L."""
    s0 = np.zeros((B * N, N), np.float32)
    sl = np.zeros((B * N, N), np.float32)
    for c in range(NCORES):
        b, h = c // 2, c % 2
        rows = slice(b * N + h * SH, b * N + h * SH + SH)
        s0[rows] = results[c]["s0o"]
        sl[rows] = results[c]["slo"]
    return s0, sl


def kernel(**inputs):
    in_maps, kp, bm2 = prepare(**inputs)
    nc = _build(kp, bm2)
    res = bass_utils.run_bass_kernel_spmd(nc, in_maps,
                                          core_ids=list(range(NCORES)))
    return assemble(res.results)

